# revision 10
# baseline (speedup 1.0000x reference)
"""Trainium2 Bass kernel for a 2-layer bidirectional LSTM char model (B=32,
T=1024, EMB=128, HID=256, OUT=5).

kernel(**inputs) takes the FULL unsharded inputs, returns FULL [B,T,5] f32
logits. Data-parallel over batch on 8 NeuronCores (4 examples/core); each
core runs all four scans (2 layers x 2 dirs), fw/bw interleaved per step so
engines ping-pong between the two independent recurrence chains.

v2 step structure (per direction-step):
  - z[128,32] f32 PSUM = I @ xp_chunk (start=True; folds the xp add into
    TensorE) + 16 accumulating W_h matmuls (8 M-tiles x 2 K-tiles, bf16 FWL)
  - ONE activation for all four gates: u = tanh(0.5*z). Host pre-doubles the
    j-gate columns of W/b so u_j = tanh(z_j); for i/f/o, sig(x)=(tanh(x/2)+1)/2
  - cell kept as s = 2c: tmp=(u_i+1)*u_j; s=(u_f+1)*s; s=0.5*s+tmp  (three
    fused scalar_tensor_tensor DVE ops); th = tanh(0.5*s) = tanh(c)
  - hist stores h' = 2h = (u_o+1)*th (one stt); downstream weights (W_h,
    layer-1 W_x, out_W) are pre-halved so the 2x cancels exactly
  - bw sequence masking is folded into xp: -100 added to the i/o gate columns
    of masked (t,b) positions during the bulk xp phase (scalar_tensor_tensor
    with the bias add), so masked steps give (u_i+1)=(u_o+1)=0 => h=0, c=0

Layouts (per core, BL=4):
  - units on partitions, (time, batch) on free dim; m-tile gate order [i,f,o,j]
  - histories hist[l][d]: [128, (T+1)*8] bf16, col = slot*8 + half*4 + b;
    fw writes slot t+1 / reads slot t; bw scans t downward, writes slot t /
    reads slot t+1; slot 0 (fw) and slot T (bw) are the zero init states
  - input projections XP = x @ W_x + b (+mask penalty) precomputed in bulk to
    DRAM scratch [128, T*32] bf16 (col = t*32 + m*4 + b), streamed back in
    CH-step chunks
  - logits: (concat(fw1,bw1) @ out_W) * mask + out_b, output [5, T*BL] f32
"""

import os
import numpy as np
import ml_dtypes

B, VOCAB, EMB, HID, OUT = 32, 256, 128, 256, 5
T_FULL = 1024
FORGET_BIAS = 1.0
NCORES = 8
BL = B // NCORES  # 4
CH = 64

bf16 = ml_dtypes.bfloat16
_cache = {}


def _tile_lhsT(W, nk, nm):
    """[K=nk*128, M=nm*128] -> [128, nk*nm*128], col block (k*nm+m)."""
    return np.ascontiguousarray(
        W.reshape(nk, 128, nm, 128).transpose(1, 0, 2, 3).reshape(128, nk * nm * 128)
    )


def _patch_tile_drain(tile_mod, mybir):
    """Pinned walrus rejects >1 sync wait on a Drain; split extras onto NOPs."""
    if getattr(tile_mod, "_drain_patched", False):
        return

    def _drain_and_barrier(self, tick_clock, wait_clock):
        nc = self.nc
        drain_inst = nc.sync.drain()
        wait_clock.add_sem_waits(
            drain_inst.ins, tile_mod.ScopedClock({None: tick_clock.global_clock})
        )
        si = drain_inst.ins.sync_info
        if si is not None and len(si.on_wait) > 1:
            waits = list(si.on_wait)
            drain_inst.ins.sync_info = mybir.SyncInfo(
                on_wait=waits[:1], on_update=list(si.on_update)
            )
            for w in waits[1:]:
                nop = nc.sync.nop(nofuse=True, hint="drain_wait_split")
                nop.ins.sync_info = mybir.SyncInfo(on_wait=[w], on_update=[])
        nc.all_engine_barrier()
        assert self.sems is not None
        popped = nc._tile_sem_poison_stack.pop()
        assert popped is self._sem_poison
        nc.clear_and_free_semaphores(list(self.sems.allocated().values()))
        nc.all_engine_barrier()

    tile_mod.TileContext._drain_and_barrier = _drain_and_barrier
    tile_mod._drain_patched = True


def _patch_compiler_wait_split():
    """Pinned walrus accepts only 1 sync wait per instruction encoding slot
    it has available; rewrite the BIR before compiling so every instruction
    carries at most 1 wait, extras moved to preceding same-engine NoOps."""
    import json
    import concourse.bass_utils as bu
    import concourse.bass2jax as b2j

    if getattr(bu, "_wsplit_patched", False):
        return
    orig = bu.compile_bir_kernel

    def fix_block(bb, ctr):
        out = []
        for inst in bb.get("instructions", []):
            for blk in inst.get("blocks") or []:
                fix_block(blk, ctr)
            si = inst.get("sync_info")
            if si:
                ow = si.get("on_wait") or []
                if len(ow) > 1:
                    for w in ow[:-1]:
                        ctr[0] += 1
                        out.append(
                            {
                                "debug": inst.get("debug", 0),
                                "engine": inst["engine"],
                                "ins": [],
                                "name": f"wsplit-{ctr[0]}",
                                "opcode": "NoOp",
                                "outs": [],
                                "text_hint": "wsplit",
                                "sync_info": {"on_wait": [w], "on_update": []},
                            }
                        )
                    si["on_wait"] = [ow[-1]]
            out.append(inst)
        bb["instructions"] = out

    def wrapped(bir_json, tmpdir, neff_name="file.neff"):
        b = json.loads(bir_json)
        ctr = [0]
        for f in b.get("functions", []):
            for bb in f.get("blocks", []):
                fix_block(bb, ctr)
        return orig(json.dumps(b).encode(), tmpdir, neff_name)

    bu.compile_bir_kernel = wrapped
    b2j.compile_bir_kernel = wrapped

    if os.environ.get("LDW_OPT", "0") == "1":
        orig_run = bu.run_command

        def run_patched(argv, **kw):
            argv = [
                "--enable-ldw-opt=true" if a == "--enable-ldw-opt=false" else a
                for a in argv
            ]
            return orig_run(argv, **kw)

        bu.run_command = run_patched
    bu._wsplit_patched = True


def _build(T):
    import concourse.bass as bass
    import concourse.mybir as mybir
    import concourse.tile as tile

    _patch_tile_drain(tile, mybir)
    _patch_compiler_wait_split()
    f32 = mybir.dt.float32
    b16 = mybir.dt.bfloat16
    Tanh = mybir.ActivationFunctionType.Tanh
    ADD = mybir.AluOpType.add
    MULT = mybir.AluOpType.mult
    NT = T * BL

    nc = bass.Bass("TRN2", target_bir_lowering=False)

    x0T = nc.dram_tensor("x0T", [128, NT], b16, kind="ExternalInput")
    mask4 = nc.dram_tensor("mask4", [128, 4 * T], b16, kind="ExternalInput")
    maskneg = nc.dram_tensor("maskneg", [128, 4 * T], b16, kind="ExternalInput")
    ident = nc.dram_tensor("ident", [128, 128], b16, kind="ExternalInput")
    whs, wxs, biases = {}, {}, {}
    for l in range(2):
        nk = 1 if l == 0 else 4
        for d in ("f", "b"):
            whs[(l, d)] = nc.dram_tensor(f"wh{l}{d}", [128, 16 * 128], b16, kind="ExternalInput")
            wxs[(l, d)] = nc.dram_tensor(f"wx{l}{d}", [128, nk * 8 * 128], b16, kind="ExternalInput")
            biases[(l, d)] = nc.dram_tensor(f"bias{l}{d}", [128, 8], f32, kind="ExternalInput")
    outw = nc.dram_tensor("outw", [128, 20], b16, kind="ExternalInput")
    outb = nc.dram_tensor("outb", [128, 1], f32, kind="ExternalInput")
    out = nc.dram_tensor("out", [5, NT], f32, kind="ExternalOutput")

    with tile.TileContext(nc) as tc:
        with tc.tile_pool(name="persist", bufs=1) as pp, \
             tc.tile_pool(name="xpbuf", bufs=2) as xpp, \
             tc.tile_pool(name="stage", bufs=4) as sp, \
             tc.tile_pool(name="small", bufs=4) as mp, \
             tc.tile_pool(name="zps", bufs=3, space="PSUM") as zp, \
             tc.tile_pool(name="pps", bufs=2, space="PSUM") as qp, \
             tc.tile_pool(name="xpd", bufs=1, space="DRAM") as dp:

            def load(name, dram, shape, dt):
                t = pp.tile(shape, dt, tag=name, name=name)
                nc.sync.dma_start(t[:], dram[:])
                return t

            x0T_s = load("x0T", x0T, [128, NT], b16)
            mask_s = load("mask4", mask4, [128, 4 * T], b16)
            maskneg_s = load("maskneg", maskneg, [128, 4 * T], b16)
            ident_s = load("ident", ident, [128, 128], b16)
            wh_s = {k: load(f"wh{k[0]}{k[1]}", v, [128, 16 * 128], b16) for k, v in whs.items()}
            wx_s = {k: load(f"wx{k[0]}{k[1]}", v, list(v.shape), b16) for k, v in wxs.items()}
            bias_s = {k: load(f"bias{k[0]}{k[1]}", v, [128, 8], f32) for k, v in biases.items()}
            outw_s = load("outw", outw, [128, 20], b16)
            outb_s = load("outb", outb, [128, 1], f32)

            hist = {}
            for l in range(2):
                for d in ("f", "b"):
                    hist[(l, d)] = pp.tile([128, (T + 1) * 8], b16, tag=f"hist{l}{d}", name=f"hist{l}{d}")
            for l in range(2):
                nc.vector.memset(hist[(l, "f")][:, 0:8], 0.0)
                nc.vector.memset(hist[(l, "b")][:, T * 8 : T * 8 + 8], 0.0)

            xp_dram = {
                (l, d): dp.tile([128, T * 32], b16, tag=f"xp{l}{d}", name=f"xp{l}{d}")
                for l in range(2)
                for d in ("f", "b")
            }

            def hist_rhs(l, d, half, c0):
                """[128, 128t, 4b] output slice of a history for XP1/logits.
                fw output for position t is slot t+1; bw output is slot t."""
                r = hist[(l, d)].rearrange("p (s q) -> p s q", q=8)
                s0 = c0 + 1 if d == "f" else c0
                return r[:, s0 : s0 + 128, half * 4 : half * 4 + 4]

            def xp_phase(l, d):
                # xp DRAM layout is M-MAJOR: col = m*(T*4) + t*4 + b, so each
                # store below is one contiguous 1KB run per partition (the
                # old t-major layout produced 16-byte strided runs => ~4M DMA
                # descriptors that saturated all 16 queues).
                nk = 1 if l == 0 else 4
                wx_t = wx_s[(l, d)]
                xp_r = xp_dram[(l, d)].rearrange("p (m s q) -> p m s q", m=8, q=4)
                for c0 in range(0, T, 128):
                    for m in range(8):
                        ps = qp.tile([128, 512], f32, tag="proj_ps")
                        for k in range(nk):
                            if l == 0:
                                rhs = x0T_s[:, c0 * 4 : c0 * 4 + 512]
                            else:
                                rhs = hist_rhs(0, "f" if k < 2 else "b", k % 2, c0)
                            nc.tensor.matmul(
                                ps[:],
                                wx_t[:, (k * 8 + m) * 128 : (k * 8 + m + 1) * 128],
                                rhs,
                                start=(k == 0),
                                stop=(k == nk - 1),
                            )
                        st = sp.tile([128, 512], b16, tag="xp_st")
                        if d == "b" and m in (0, 1, 4, 5):
                            # add bias then the -100 mask penalty (i/o gates)
                            nc.vector.scalar_tensor_tensor(
                                st[:], ps[:], bias_s[(l, d)][:, m : m + 1],
                                maskneg_s[:, c0 * 4 : c0 * 4 + 512],
                                op0=ADD, op1=ADD,
                            )
                        else:
                            nc.vector.tensor_scalar_add(st[:], ps[:], bias_s[(l, d)][:, m : m + 1])
                        nc.sync.dma_start(xp_r[:, m, c0 : c0 + 128, :], st[:])

            def step(l, d, t, xp_tile, i, s_in, s_out):
                """One LSTM step; reads/writes hist[(l,d)], s tiles, xp chunk."""
                h = hist[(l, d)]
                if d == "f":
                    r_off, w_off = t * 8, (t + 1) * 8
                else:
                    r_off, w_off = (t + 1) * 8, t * 8
                z = zp.tile([128, 32], f32, tag=f"z{d}")
                # xp lands first via identity matmul (start=True covers all 32);
                # chunk tile is m-major [128, 8m, CH, 4b] -> step slice [8, 4]
                xr = xp_tile.rearrange("p (m s q) -> p m s q", m=8, q=4)
                nc.tensor.matmul(
                    z[:, 0:32], ident_s[:, 0:128],
                    xr[:, :, i, :],
                    start=True, stop=False,
                )
                wh_t = wh_s[(l, d)]
                for m in range(8):
                    for k in range(2):
                        nc.tensor.matmul(
                            z[:, 4 * m : 4 * m + 4],
                            wh_t[:, (k * 8 + m) * 128 : (k * 8 + m + 1) * 128],
                            h[:, r_off + 4 * k : r_off + 4 * k + 4],
                            start=False,
                            stop=(k == 1),
                        )
                u = sp.tile([128, 32], f32, tag=f"u{d}")
                nc.scalar.activation(u[:], z[:], Tanh, scale=0.5)
                tmp = mp.tile([128, 8], f32, tag=f"tmp{d}")
                nc.vector.scalar_tensor_tensor(
                    tmp[:], u[:, 0:8], 1.0, u[:, 24:32], op0=ADD, op1=MULT
                )
                tm2 = mp.tile([128, 8], f32, tag=f"tm2{d}")
                nc.vector.scalar_tensor_tensor(
                    tm2[:], u[:, 8:16], 1.0, s_in[:], op0=ADD, op1=MULT
                )
                nc.vector.scalar_tensor_tensor(
                    s_out[:], tm2[:], 0.5, tmp[:], op0=MULT, op1=ADD
                )
                th = mp.tile([128, 8], f32, tag=f"th{d}")
                nc.scalar.activation(th[:], s_out[:], Tanh, scale=0.5)
                nc.vector.scalar_tensor_tensor(
                    h[:, w_off : w_off + 8], u[:, 16:24], 1.0, th[:],
                    op0=ADD, op1=MULT,
                )

            cs = {}
            for l in range(2):
                for d in ("f", "b"):
                    for par in (0, 1):
                        cs[(l, d, par)] = pp.tile(
                            [128, 8], f32, tag=f"c{l}{d}{par}", name=f"c{l}{d}{par}"
                        )

            for l in range(2):
                xp_phase(l, "f")
                xp_phase(l, "b")
                nc.vector.memset(cs[(l, "f", 0)][:], 0.0)
                nc.vector.memset(cs[(l, "b", 0)][:], 0.0)
                xpf_r = xp_dram[(l, "f")].rearrange("p (m s q) -> p m s q", m=8, q=4)
                xpb_r = xp_dram[(l, "b")].rearrange("p (m s q) -> p m s q", m=8, q=4)

                def fetch(c0):
                    xf = xpp.tile([128, CH * 32], b16, tag="xf")
                    nc.sync.dma_start(
                        xf.rearrange("p (m s q) -> p m s q", m=8, q=4)[:],
                        xpf_r[:, :, c0 : c0 + CH, :],
                    )
                    xb = xpp.tile([128, CH * 32], b16, tag="xb")
                    bw_lo = T - c0 - CH
                    nc.sync.dma_start(
                        xb.rearrange("p (m s q) -> p m s q", m=8, q=4)[:],
                        xpb_r[:, :, bw_lo : bw_lo + CH, :],
                    )
                    return xf, xb

                cur = fetch(0)
                for c0 in range(0, T, CH):
                    nxt = fetch(c0 + CH) if c0 + CH < T else None
                    xf, xb = cur
                    for i in range(CH):
                        g = c0 + i
                        pi, po = g % 2, (g + 1) % 2
                        step(l, "f", g, xf, i,
                             cs[(l, "f", pi)], cs[(l, "f", po)])
                        step(l, "b", T - 1 - g, xb, CH - 1 - i,
                             cs[(l, "b", pi)], cs[(l, "b", po)])
                    cur = nxt

            # ---- logits ----
            mask_r = mask_s.rearrange("p (s q) -> p s q", q=4)
            for c0 in range(0, T, 128):
                ps = qp.tile([128, 512], f32, tag="proj_ps")
                for k in range(4):
                    rhs = hist_rhs(1, "f" if k < 2 else "b", k % 2, c0)
                    nc.tensor.matmul(
                        ps[:5, :],
                        outw_s[:, k * 5 : k * 5 + 5],
                        rhs,
                        start=(k == 0),
                        stop=(k == 3),
                    )
                lg = sp.tile([5, 512], f32, tag="lg")
                nc.vector.tensor_mul(lg[:], ps[:5, :], mask_r[:5, c0 : c0 + 128, 0:4])
                nc.vector.tensor_scalar_add(lg[:], lg[:], outb_s[:5, 0:1])
                nc.sync.dma_start(out[:, c0 * 4 : c0 * 4 + 512], lg[:])

    return nc


last_results = None


def kernel(**inputs):
    global last_results
    T = int(os.environ.get("KERNEL_T", T_FULL))
    from concourse.bass_utils import run_bass_kernel_spmd

    tokens = np.asarray(inputs["tokens"])[:, :T]
    lengths = np.clip(np.asarray(inputs["lengths"]), 0, T)
    emb = np.asarray(inputs["emb"], dtype=np.float32)

    if T not in _cache:
        _cache[T] = _build(T)
    nc = _cache[T]

    # ---- host-side retiling (shared across cores) ----
    # gate m-tile order [i, f, o, j]; j columns pre-doubled (tanh-only gate
    # trick); W_h rows and layer-1 W_x rows pre-halved (hist stores 2h).
    perm = np.concatenate([
        np.arange(0, 256),        # i
        np.arange(512, 768),      # f
        np.arange(768, 1024),     # o
        np.arange(256, 512),      # j
    ])
    shared = {}
    for l in range(2):
        D = EMB if l == 0 else 2 * HID
        nk = D // 128
        for d, pre in (("f", "fw"), ("b", "bw")):
            W = np.asarray(inputs[f"{pre}_W{l}"], dtype=np.float32).copy()
            bias = np.asarray(inputs[f"{pre}_b{l}"], dtype=np.float32).copy()
            bias[2 * HID : 3 * HID] += FORGET_BIAS
            W[:, HID : 2 * HID] *= 2.0          # j-gate doubling
            bias[HID : 2 * HID] *= 2.0
            Wh = W[D:] * 0.5                     # h' = 2h stored
            Wx = W[:D] * (0.5 if l == 1 else 1.0)
            shared[f"wh{l}{d}"] = _tile_lhsT(Wh[:, perm], 2, 8).astype(bf16)
            shared[f"wx{l}{d}"] = _tile_lhsT(Wx[:, perm], nk, 8).astype(bf16)
            shared[f"bias{l}{d}"] = np.ascontiguousarray(
                bias[perm].reshape(8, 128).T
            ).astype(np.float32)
    shared["outw"] = np.ascontiguousarray(
        (np.asarray(inputs["out_W"], dtype=np.float32) * 0.5)
        .reshape(4, 128, 5)
        .transpose(1, 0, 2)
        .reshape(128, 20)
    ).astype(bf16)
    ob = np.zeros((128, 1), np.float32)
    ob[:5, 0] = np.asarray(inputs["out_b"], dtype=np.float32)
    shared["outb"] = ob
    shared["ident"] = np.eye(128, dtype=np.float32).astype(bf16)

    in_maps = []
    for ci in range(NCORES):
        bs = slice(ci * BL, (ci + 1) * BL)
        x0 = emb[tokens[bs]]  # [BL, T, 128]
        x0T = np.ascontiguousarray(x0.transpose(2, 1, 0).reshape(128, T * BL)).astype(bf16)
        mvec = (np.arange(T)[:, None] < lengths[bs][None, :])  # [T, BL]
        m4 = mvec.astype(bf16).reshape(1, T * 4)
        mask4 = np.ascontiguousarray(np.broadcast_to(m4, (128, T * 4)))
        mneg = ((~mvec).astype(np.float32) * -100.0).astype(bf16).reshape(1, T * 4)
        maskneg = np.ascontiguousarray(np.broadcast_to(mneg, (128, T * 4)))
        im = dict(shared)
        im["x0T"] = x0T
        im["mask4"] = mask4
        im["maskneg"] = maskneg
        in_maps.append(im)

    res = run_bass_kernel_spmd(nc, in_maps, core_ids=list(range(NCORES)))
    last_results = res
    outs = []
    for ci in range(NCORES):
        o = res.results[ci]["out"]  # [5, T*BL]
        outs.append(o.reshape(5, T, BL).transpose(2, 1, 0))  # [BL, T, 5]
    return np.concatenate(outs, axis=0).astype(np.float32)
